# revision 19
# baseline (speedup 1.0000x reference)
"""CTRNN (6 unfolds) Trainium2 Bass kernel, data-parallel over 8 NeuronCores.

Math (per reference):
    w_x = fc_w[:, :512]; w_h = fc_w[:, 512:]
    xw  = x @ w_x^T + b
    repeat 6x:  f = tanh(xw + h @ w_h^T);  h = 0.9*h + 0.1*f

Device layout: everything transposed ([feature, batch]); per core batch
shard of 2048.  State kept rescaled (H_t = h_t / 0.9^t) in bf16 so the
update is one fused DVE op in 16-bit 2x mode.  Recurrent matmuls run in
fp8 (e4m3) with perf_mode=DoubleRow: weights host-quantized (scale S_W),
state quantized on-chip each step by the scalar engine (Copy activation,
scale = S_H * 0.9^t, fp8 out).  The loop-invariant xw (+bias, folded in
during the psum->sbuf copy) runs once in fp32r.  The final 0.9^6 rescale
of the state happens on the host during gather; the kernel stores the
raw bf16 state.  Engine split per unfold: PE=matmuls, DVE=descale-stt +
state update, ACT=tanh + fp8 quantize, POOL=DMA issue only (Q7
elementwise is ~60us/op on this image - never put compute there).
"""

import numpy as np
import ml_dtypes
from contextlib import ExitStack

import concourse.bass as bass
import concourse.tile as tile
import concourse.mybir as mybir
from concourse.bass_utils import run_bass_kernel_spmd


def _patch_tile_drain():
    """The walrus build in this image encodes at most one sync-wait on a
    Drain CTRL instruction; Tile's kernel-tail drain attaches one wait per
    outstanding proc and fails codegen ("Too many sync wait commands").
    Spread those waits across single-wait SP nops, then emit a bare drain."""
    if getattr(tile.TileContext, "_drain_split_patched", False):
        return
    from concourse.vector_clock import ScopedClock

    def _drain_and_barrier(self, tick_clock, wait_clock):
        nc = self.nc
        collector = nc.sync.nop(nofuse=True)
        wait_clock.add_sem_waits(
            collector.ins, ScopedClock({None: tick_clock.global_clock})
        )
        waits = list(collector.ins.sync_info.on_wait)
        del collector.ins.sync_info.on_wait[1:]
        for w in waits[1:]:
            nop = nc.sync.nop(nofuse=True)
            if nop.ins.sync_info is None:
                nop.ins.sync_info = mybir.SyncInfo(on_wait=[], on_update=[])
            nop.ins.sync_info.on_wait.append(w)
        nc.sync.drain()
        nc.all_engine_barrier()
        assert self.sems is not None
        popped = nc._tile_sem_poison_stack.pop()
        assert popped is self._sem_poison
        nc.clear_and_free_semaphores(list(self.sems.allocated().values()))
        nc.all_engine_barrier()

    tile.TileContext._drain_and_barrier = _drain_and_barrier
    tile.TileContext._drain_split_patched = True


_patch_tile_drain()


def _split_excess_waits_json(bir_json):
    """This image's walrus encodes at most ONE sync-wait per instruction
    (setupSyncWait: "Too many sync wait commands").  Tile attaches as many
    waits as deps require.  Hoist all but one wait of each instruction onto
    injected NoOps, placed just before it on the same engine."""
    import json as _json

    js = _json.loads(bir_json)
    n_split = 0
    for fn in js["functions"]:
        for blk in fn["blocks"]:
            out_insts = []
            for inst in blk["instructions"]:
                si = inst.get("sync_info") or {}
                ow = si.get("on_wait") or []
                if len(ow) > 1:
                    for w in ow[:-1]:
                        n_split += 1
                        nop = {
                            "name": f"I-ws{n_split}",
                            "opcode": "NoOp",
                            "engine": inst["engine"],
                            "ins": [],
                            "outs": [],
                            "sync_info": {"on_update": [], "on_wait": [w]},
                        }
                        if "debug" in inst:
                            nop["debug"] = inst["debug"]
                        out_insts.append(nop)
                    si["on_wait"] = [ow[-1]]
                out_insts.append(inst)
            blk["instructions"] = out_insts
    return _json.dumps(js).encode()


def _patch_compile_for_wait_cap():
    import concourse.bass_utils as _bu

    if getattr(_bu, "_wait_split_patched", False):
        return
    _orig = _bu._compile_bir_impl

    def _impl(bir_json, *args, **kwargs):
        return _orig(_split_excess_waits_json(bir_json), *args, **kwargs)

    _bu._compile_bir_impl = _impl
    _bu._wait_split_patched = True


_patch_compile_for_wait_cap()

B, D_IN, D_H = 16384, 512, 1024
N_CORES = 8
BS = B // N_CORES            # 2048 batch rows per core
UNFOLDS = 6
DT = 0.1
DECAY = 0.9                  # 1 - DT/TAU
CH = 512                     # batch chunk (matmul moving free dim)
NCH = BS // CH               # 4 chunks per core
KB = D_H // 128              # 8 hidden-dim k-blocks
KX = D_IN // 128             # 4 input-dim k-blocks
F32 = mybir.dt.float32
F32R = mybir.dt.float32r
FP8 = mybir.dt.float8e4
BF16 = mybir.dt.bfloat16
E4NP = ml_dtypes.float8_e4m3  # TRN FP8_EXP4-compatible (max 240)
BF16NP = ml_dtypes.bfloat16

S_W = 1024.0                 # host-side scale on w_h before fp8 quantize
S_H = 16.0                   # on-chip scale on h before fp8 quantize


def build_nc() -> bass.Bass:
    nc = bass.Bass()
    xT = nc.dram_tensor("xT", [D_IN, BS], BF16, kind="ExternalInput")
    hT = nc.dram_tensor("hT", [D_H, BS], BF16, kind="ExternalInput")
    wxT = nc.dram_tensor("wxT", [D_IN, D_H], BF16, kind="ExternalInput")
    whq = nc.dram_tensor("whq", [D_H, D_H], FP8, kind="ExternalInput")
    ident = nc.dram_tensor("ident", [128, 128], BF16, kind="ExternalInput")
    bias = nc.dram_tensor("bias", [128, KB], F32, kind="ExternalInput")
    out = nc.dram_tensor("out", [D_H, BS], BF16, kind="ExternalOutput")

    MUL = mybir.AluOpType.mult
    ADD = mybir.AluOpType.add
    Tanh = mybir.ActivationFunctionType.Tanh
    Ident = mybir.ActivationFunctionType.Identity

    with tile.TileContext(nc) as tc, ExitStack() as ctx:
        persist = ctx.enter_context(tc.tile_pool(name="persist", bufs=1))
        psum_pool = ctx.enter_context(tc.tile_pool(name="psum", bufs=4, space="PSUM"))

        # --- persistent SBUF state ---
        # H (rescaled h) bf16, one 3D tile per batch chunk: [128, KB, CH]
        h_sb = [
            persist.tile([128, KB, CH], BF16, name=f"h_sb{c}", tag=f"h_sb{c}")
            for c in range(NCH)
        ]
        # fp8 copies of S_H*h_t (physical h), double-buffered by step parity
        hq_sb = [
            [
                persist.tile([128, KB, CH], FP8, name=f"hq{pr}_{c}", tag=f"hq{pr}_{c}")
                for c in range(NCH)
            ]
            for pr in range(2)
        ]
        # w_h^T fp8 (host-quantized, scaled by S_W): [128, KB, D_H]
        wh_sb = persist.tile([128, KB, D_H], FP8, name="wh_sb", tag="wh_sb")
        id_sb = persist.tile([128, 128], BF16, name="id_sb", tag="id_sb")
        b_sb = persist.tile([128, KB], F32, name="b_sb", tag="b_sb")
        # xw + bias resident in SBUF as bf16, same [128, KB, CH] layout as h
        xw_sb = [
            persist.tile([128, KB, CH], BF16, name=f"xw_sb{c}", tag=f"xw_sb{c}")
            for c in range(NCH)
        ]

        nc.sync.dma_start(out=b_sb[:], in_=bias[:, :])
        nc.sync.dma_start(out=id_sb[:], in_=ident[:, :])

        # --- phase 1: xw = x @ w_x^T + b (fp32r), kept in SBUF ---
        with tc.tile_pool(name="xpre", bufs=1) as xpool, \
             tc.tile_pool(name="wxpre", bufs=1) as wxpool:
            wx_sb = wxpool.tile([128, KX * D_H], BF16, name="wx_sb", tag="wx_sb")
            x_sbs = [
                xpool.tile([128, KX * CH], BF16, name="x_sb", tag=f"x_sb{c}")
                for c in range(NCH)
            ]
            # per-k-block loads; a single DMA queue streams at only
            # ~45-90 GB/s, so the transfers gating the first matmuls (wx
            # k-block 0, x0 k-block 0) are split across parallel queues,
            # and wx k-blocks alternate sync/scalar.
            nc.sync.dma_start(
                out=wx_sb[:, 0:512],
                in_=wxT[0:128, 0:512],
            )
            nc.scalar.dma_start(
                out=wx_sb[:, 512:D_H],
                in_=wxT[0:128, 512:D_H],
            )
            nc.gpsimd.dma_start(
                out=x_sbs[0][:, 0:CH],
                in_=xT[0:128, 0:CH],
            )
            for kb in range(1, KX):
                eng = nc.sync if kb % 2 == 1 else nc.scalar
                eng.dma_start(
                    out=wx_sb[:, kb * D_H:(kb + 1) * D_H],
                    in_=wxT[kb * 128:(kb + 1) * 128, :],
                )
                nc.gpsimd.dma_start(
                    out=x_sbs[0][:, kb * CH:(kb + 1) * CH],
                    in_=xT[kb * 128:(kb + 1) * 128, 0:CH],
                )
            for c in range(1, NCH):
                nc.scalar.dma_start(
                    out=x_sbs[c][:].rearrange("p (kb c) -> p kb c", c=CH),
                    in_=xT[:, c * CH:(c + 1) * CH].rearrange("(kb p) c -> p kb c", p=128),
                )
            # recurrent-phase loads: fp8 weights (1MB) then bf16 h chunks;
            # each h chunk is quantized to fp8 on DVE as soon as it lands.
            nc.scalar.dma_start(
                out=wh_sb[:],
                in_=whq[:, :].rearrange("(jb p) h -> p jb h", p=128),
            )
            for hc in range(NCH):
                nc.gpsimd.dma_start(
                    out=h_sb[hc][:],
                    in_=hT[:, hc * CH:(hc + 1) * CH].rearrange("(jb p) c -> p jb c", p=128),
                )
                nc.vector.tensor_scalar_mul(hq_sb[0][hc][:], h_sb[hc][:], S_H)
            # chunk 0: kb-outer across all 8 psum banks, so matmuls
            # start as soon as each wx k-block's DMA lands (no mid-group
            # stalls while the head loads stream in).
            ps0 = [
                psum_pool.tile([128, 2, CH], F32, name="ps0", tag="ps")
                for _ in range(KB // 2)
            ]
            for kb in range(KX):
                for p in range(KB):
                    nc.tensor.matmul(
                        ps0[p // 2][:, p % 2, :],
                        wx_sb[:, kb * D_H + p * 128: kb * D_H + (p + 1) * 128],
                        x_sbs[0][:, kb * CH:(kb + 1) * CH],
                        start=(kb == 0),
                        stop=(kb == KX - 1),
                    )
            for p in range(KB):
                # xw_pre = psum*(S_W*S_H) + b_pre on ACT (bias comes
                # pre-scaled from the host); pre-scaling lets the unfold
                # psum group add xw via an identity matmul with the tanh
                # descale folded into the activation input scale.
                nc.scalar.activation(
                    xw_sb[0][:, p, :], ps0[p // 2][:, p % 2, :], Ident,
                    bias=b_sb[:, p:p + 1], scale=float(S_W * S_H),
                )
            for c in range(1, NCH):
                x_sb = x_sbs[c]
                for p in range(KB):
                    ps1 = psum_pool.tile([128, 2, CH], F32, name="ps1", tag="ps")
                    for kb in range(KX):
                        nc.tensor.matmul(
                            ps1[:, 0, :],
                            wx_sb[:, kb * D_H + p * 128: kb * D_H + (p + 1) * 128],
                            x_sb[:, kb * CH:(kb + 1) * CH],
                            start=(kb == 0),
                            stop=(kb == KX - 1),
                        )
                    nc.scalar.activation(
                        xw_sb[c][:, p, :], ps1[:, 0, :], Ident,
                        bias=b_sb[:, p:p + 1], scale=float(S_W * S_H),
                    )

        # --- phase 2: unfold loop ---
        # hq holds S_H * h_t (physical h); psum = xw_pre + S_W*S_H*(W^T h)
        # via an identity matmul first in each accumulation group, so the
        # descale is just the tanh input scale (no DVE stt at all).
        fpool = ctx.enter_context(tc.tile_pool(name="fpool", bufs=2))
        zpool = ctx.enter_context(tc.tile_pool(name="zpool", bufs=3))
        alpha = 1.0 / (S_W * S_H)
        sigma = 1.0  # SBUF h_sb holds H_t = h_t / sigma
        deferred = []  # closures emitting DVE tail pieces (update/quantize)

        def flush(n_keep):
            while len(deferred) > n_keep:
                deferred.pop(0)()

        for t in range(UNFOLDS):
            last = t == UNFOLDS - 1
            par, nxt = t % 2, (t + 1) % 2
            upd = DT / (sigma * DECAY)    # coefficient on f for the H update
            qscale = S_H * sigma * DECAY  # h_{t+1} = sigma*0.9*H_{t+1}
            for c in range(NCH):
                f_ch = fpool.tile(
                    [128, KB, CH], BF16, name="f_ch", tag="f_ch", bufs=2
                )
                for q in range(KB // 2):
                    # groups 2-3: xw enters the psum via an identity matmul
                    # (on PE); groups 0-1: plain DVE add, balancing PE/DVE.
                    # q0/q1 so the adds sit at the DVE queue head for the
                    # chunk - their psum tiles are WAR-needed by the next
                    # chunk's q0/q1 MMs.
                    on_pe = q >= 2
                    ps = psum_pool.tile([128, 2, CH], F32, name="ps", tag="ps")
                    for i in range(2):
                        p = 2 * q + i
                        if on_pe:
                            nc.tensor.matmul(
                                ps[:, i, :], id_sb[:], xw_sb[c][:, p, :],
                                start=True, stop=False,
                            )
                        for jj in range(KB // 2):
                            nc.tensor.matmul(
                                ps[:, i, :],
                                wh_sb[:, 2 * jj:2 * jj + 2, p * 128:(p + 1) * 128],
                                hq_sb[par][c][:, 2 * jj:2 * jj + 2, :],
                                start=(not on_pe and jj == 0),
                                stop=(jj == KB // 2 - 1),
                                perf_mode=mybir.MatmulPerfMode.DoubleRow,
                            )
                    if on_pe:
                        nc.scalar.activation(
                            f_ch[:, 2 * q:2 * q + 2, :], ps[:], Tanh,
                            scale=float(alpha),
                        )
                    else:
                        zt = zpool.tile([128, 2, CH], BF16, name="zt",
                                        tag="zt", bufs=3)
                        nc.vector.tensor_tensor(
                            zt[:], ps[:], xw_sb[c][:, 2 * q:2 * q + 2, :], ADD
                        )
                        nc.scalar.activation(
                            f_ch[:, 2 * q:2 * q + 2, :], zt[:], Tanh,
                            scale=float(alpha),
                        )
                    if last or q % 2 == 1:
                        # tail pieces for the slice just finished; defer so
                        # they land behind the next groups' work on DVE.  On
                        # the last step go per-group so the final chain after
                        # the last matmul is as short as possible.
                        k = q // 2
                        sl = slice(2 * q, 2 * q + 2) if last else slice(4 * k, 4 * k + 4)
                        def piece(c=c, sl=sl, k=k, upd=upd, qscale=qscale,
                                  f_ch=f_ch, par_nxt=nxt, last=last):
                            nc.vector.scalar_tensor_tensor(
                                h_sb[c][:, sl, :], f_ch[:, sl, :], float(upd),
                                h_sb[c][:, sl, :], op0=MUL, op1=ADD,
                            )
                            if last:
                                # raw half-chunk state out as soon as it's
                                # final; host multiplies by 0.9^6
                                nc.sync.dma_start(
                                    out=out[
                                        sl.start * 128:sl.stop * 128,
                                        c * CH:(c + 1) * CH,
                                    ].rearrange("(jb p) c -> p jb c", p=128),
                                    in_=h_sb[c][:, sl, :],
                                )
                            elif k == 0:
                                nc.vector.tensor_scalar_mul(
                                    hq_sb[par_nxt][c][:, sl, :],
                                    h_sb[c][:, sl, :], float(qscale),
                                )
                            else:
                                nc.scalar.activation(
                                    hq_sb[par_nxt][c][:, sl, :],
                                    h_sb[c][:, sl, :], Ident,
                                    scale=float(qscale),
                                )
                        deferred.append(piece)
                    if q >= 2:
                        # don't flush between the q0/q1 DVE adds - they must
                        # stay at the DVE queue head to free psum quickly
                        flush(1 if last else 2)
            sigma *= DECAY
        flush(0)
    return nc


_NC_CACHE = {}


def _get_nc() -> bass.Bass:
    if "nc" not in _NC_CACHE:
        _NC_CACHE["nc"] = build_nc()
    return _NC_CACHE["nc"]


def make_in_maps(x, h, fc_w, fc_b):
    x = np.asarray(x, dtype=np.float32)
    h = np.asarray(h, dtype=np.float32)
    fc_w = np.asarray(fc_w, dtype=np.float32)
    fc_b = np.asarray(fc_b, dtype=np.float32)
    xT = np.ascontiguousarray(x.T.astype(BF16NP))     # [D_IN, B] bf16
    hT = np.ascontiguousarray(h.T.astype(BF16NP))     # [D_H, B] bf16
    wxT = np.ascontiguousarray(fc_w[:, :D_IN].T.astype(BF16NP))  # [D_IN, D_H] bf16
    whT = np.ascontiguousarray(fc_w[:, D_IN:].T)      # [D_H, D_H]
    whq = np.clip(whT * S_W, -240.0, 240.0).astype(E4NP)
    bias = np.ascontiguousarray(fc_b.reshape(KB, 128).T) * (S_W * S_H)  # [128, KB]
    identity = np.eye(128, dtype=np.float32).astype(BF16NP)
    in_maps = []
    for i in range(N_CORES):
        sl = slice(i * BS, (i + 1) * BS)
        in_maps.append({
            "xT": np.ascontiguousarray(xT[:, sl]),
            "hT": np.ascontiguousarray(hT[:, sl]),
            "wxT": wxT,
            "whq": whq,
            "ident": identity,
            "bias": bias,
        })
    return in_maps


def gather_out(results):
    outT = np.concatenate([results[i]["out"] for i in range(N_CORES)], axis=1)
    # kernel stores the raw rescaled state H_6 = h_6 / 0.9^6 in bf16
    return np.ascontiguousarray(outT.T).astype(np.float32) * (DECAY ** UNFOLDS)


def kernel(x, h, fc_w, fc_b):
    nc = _get_nc()
    in_maps = make_in_maps(x, h, fc_w, fc_b)
    res = run_bass_kernel_spmd(nc, in_maps, list(range(N_CORES)))
    out = gather_out(res.results)
    return (out, out)


if __name__ == "__main__":
    rng = np.random.default_rng(0)
    x = rng.standard_normal((B, D_IN), dtype=np.float32)
    h = rng.standard_normal((B, D_H), dtype=np.float32)
    fc_w = rng.standard_normal((D_H, D_IN + D_H), dtype=np.float32) / np.sqrt(D_IN + D_H)
    fc_b = np.zeros((D_H,), dtype=np.float32)
    o, _ = kernel(x, h, fc_w, fc_b)
    print(o.shape, o.dtype)


# revision 20
# speedup vs baseline: 1.1409x; 1.1409x over previous
"""CTRNN (6 unfolds) Trainium2 Bass kernel, data-parallel over 8 NeuronCores.

Math (per reference):
    w_x = fc_w[:, :512]; w_h = fc_w[:, 512:]
    xw  = x @ w_x^T + b
    repeat 6x:  f = tanh(xw + h @ w_h^T);  h = 0.9*h + 0.1*f

Device layout: everything transposed ([feature, batch]); per core batch
shard of 2048.  State kept rescaled (H_t = h_t / 0.9^t) in bf16 so the
update is one fused DVE op in 16-bit 2x mode.  Recurrent matmuls run in
fp8 (e4m3) with perf_mode=DoubleRow: weights host-quantized (scale S_W),
state quantized on-chip each step by the scalar engine (Copy activation,
scale = S_H * 0.9^t, fp8 out).  The loop-invariant xw (+bias, folded in
during the psum->sbuf copy) runs once in fp32r.  The final 0.9^6 rescale
of the state happens on the host during gather; the kernel stores the
raw bf16 state.  Engine split per unfold: PE=matmuls, DVE=descale-stt +
state update, ACT=tanh + fp8 quantize, POOL=DMA issue only (Q7
elementwise is ~60us/op on this image - never put compute there).
"""

import numpy as np
import ml_dtypes
from contextlib import ExitStack

import concourse.bass as bass
import concourse.tile as tile
import concourse.mybir as mybir
from concourse.bass_utils import run_bass_kernel_spmd


def _patch_tile_drain():
    """The walrus build in this image encodes at most one sync-wait on a
    Drain CTRL instruction; Tile's kernel-tail drain attaches one wait per
    outstanding proc and fails codegen ("Too many sync wait commands").
    Spread those waits across single-wait SP nops, then emit a bare drain."""
    if getattr(tile.TileContext, "_drain_split_patched", False):
        return
    from concourse.vector_clock import ScopedClock

    def _drain_and_barrier(self, tick_clock, wait_clock):
        nc = self.nc
        collector = nc.sync.nop(nofuse=True)
        wait_clock.add_sem_waits(
            collector.ins, ScopedClock({None: tick_clock.global_clock})
        )
        waits = list(collector.ins.sync_info.on_wait)
        del collector.ins.sync_info.on_wait[1:]
        for w in waits[1:]:
            nop = nc.sync.nop(nofuse=True)
            if nop.ins.sync_info is None:
                nop.ins.sync_info = mybir.SyncInfo(on_wait=[], on_update=[])
            nop.ins.sync_info.on_wait.append(w)
        nc.sync.drain()
        nc.all_engine_barrier()
        assert self.sems is not None
        popped = nc._tile_sem_poison_stack.pop()
        assert popped is self._sem_poison
        nc.clear_and_free_semaphores(list(self.sems.allocated().values()))
        nc.all_engine_barrier()

    tile.TileContext._drain_and_barrier = _drain_and_barrier
    tile.TileContext._drain_split_patched = True


_patch_tile_drain()


def _split_excess_waits_json(bir_json):
    """This image's walrus encodes at most ONE sync-wait per instruction
    (setupSyncWait: "Too many sync wait commands").  Tile attaches as many
    waits as deps require.  Hoist all but one wait of each instruction onto
    injected NoOps, placed just before it on the same engine."""
    import json as _json

    js = _json.loads(bir_json)
    n_split = 0
    for fn in js["functions"]:
        for blk in fn["blocks"]:
            out_insts = []
            for inst in blk["instructions"]:
                si = inst.get("sync_info") or {}
                ow = si.get("on_wait") or []
                if len(ow) > 1:
                    for w in ow[:-1]:
                        n_split += 1
                        nop = {
                            "name": f"I-ws{n_split}",
                            "opcode": "NoOp",
                            "engine": inst["engine"],
                            "ins": [],
                            "outs": [],
                            "sync_info": {"on_update": [], "on_wait": [w]},
                        }
                        if "debug" in inst:
                            nop["debug"] = inst["debug"]
                        out_insts.append(nop)
                    si["on_wait"] = [ow[-1]]
                out_insts.append(inst)
            blk["instructions"] = out_insts
    return _json.dumps(js).encode()


def _patch_compile_for_wait_cap():
    import concourse.bass_utils as _bu

    if getattr(_bu, "_wait_split_patched", False):
        return
    _orig = _bu._compile_bir_impl

    def _impl(bir_json, *args, **kwargs):
        return _orig(_split_excess_waits_json(bir_json), *args, **kwargs)

    _bu._compile_bir_impl = _impl
    _bu._wait_split_patched = True


_patch_compile_for_wait_cap()

B, D_IN, D_H = 16384, 512, 1024
N_CORES = 8
BS = B // N_CORES            # 2048 batch rows per core
UNFOLDS = 6
DT = 0.1
DECAY = 0.9                  # 1 - DT/TAU
CH = 512                     # batch chunk (matmul moving free dim)
NCH = BS // CH               # 4 chunks per core
KB = D_H // 128              # 8 hidden-dim k-blocks
KX = D_IN // 128             # 4 input-dim k-blocks
F32 = mybir.dt.float32
F32R = mybir.dt.float32r
FP8 = mybir.dt.float8e4
BF16 = mybir.dt.bfloat16
E4NP = ml_dtypes.float8_e4m3  # TRN FP8_EXP4-compatible (max 240)
BF16NP = ml_dtypes.bfloat16

S_W = 1024.0                 # host-side scale on w_h before fp8 quantize
S_H = 16.0                   # on-chip scale on h before fp8 quantize


def build_nc() -> bass.Bass:
    nc = bass.Bass()
    xT = nc.dram_tensor("xT", [D_IN, BS], BF16, kind="ExternalInput")
    hT = nc.dram_tensor("hT", [D_H, BS], BF16, kind="ExternalInput")
    wxT = nc.dram_tensor("wxT", [D_IN, D_H], BF16, kind="ExternalInput")
    whq = nc.dram_tensor("whq", [D_H, D_H], FP8, kind="ExternalInput")
    ident = nc.dram_tensor("ident", [128, 128], BF16, kind="ExternalInput")
    bias = nc.dram_tensor("bias", [128, KB], F32, kind="ExternalInput")
    out = nc.dram_tensor("out", [D_H, BS], BF16, kind="ExternalOutput")

    MUL = mybir.AluOpType.mult
    ADD = mybir.AluOpType.add
    Tanh = mybir.ActivationFunctionType.Tanh
    Ident = mybir.ActivationFunctionType.Identity

    with tile.TileContext(nc) as tc, ExitStack() as ctx:
        persist = ctx.enter_context(tc.tile_pool(name="persist", bufs=1))
        psum_pool = ctx.enter_context(tc.tile_pool(name="psum", bufs=4, space="PSUM"))

        # --- persistent SBUF state ---
        # H (rescaled h) bf16, one 3D tile per batch chunk: [128, KB, CH]
        h_sb = [
            persist.tile([128, KB, CH], BF16, name=f"h_sb{c}", tag=f"h_sb{c}")
            for c in range(NCH)
        ]
        # fp8 copies of S_H*h_t (physical h), double-buffered by step parity
        hq_sb = [
            [
                persist.tile([128, KB, CH], FP8, name=f"hq{pr}_{c}", tag=f"hq{pr}_{c}")
                for c in range(NCH)
            ]
            for pr in range(2)
        ]
        # w_h^T fp8 (host-quantized, scaled by S_W): [128, KB, D_H]
        wh_sb = persist.tile([128, KB, D_H], FP8, name="wh_sb", tag="wh_sb")
        id_sb = persist.tile([128, 128], BF16, name="id_sb", tag="id_sb")
        b_sb = persist.tile([128, KB], F32, name="b_sb", tag="b_sb")
        # xw + bias resident in SBUF as bf16, same [128, KB, CH] layout as h
        xw_sb = [
            persist.tile([128, KB, CH], BF16, name=f"xw_sb{c}", tag=f"xw_sb{c}")
            for c in range(NCH)
        ]

        nc.sync.dma_start(out=b_sb[:], in_=bias[:, :])
        nc.sync.dma_start(out=id_sb[:], in_=ident[:, :])

        # --- phase 1: xw = x @ w_x^T + b (fp32r), kept in SBUF ---
        with tc.tile_pool(name="xpre", bufs=1) as xpool, \
             tc.tile_pool(name="wxpre", bufs=1) as wxpool:
            wx_sb = wxpool.tile([128, KX * D_H], BF16, name="wx_sb", tag="wx_sb")
            x_sbs = [
                xpool.tile([128, KX * CH], BF16, name="x_sb", tag=f"x_sb{c}")
                for c in range(NCH)
            ]
            # per-k-block loads; a single DMA queue streams at only
            # ~45-90 GB/s, so the transfers gating the first matmuls (wx
            # k-block 0, x0 k-block 0) are split across parallel queues,
            # and wx k-blocks alternate sync/scalar.
            nc.sync.dma_start(
                out=wx_sb[:, 0:512],
                in_=wxT[0:128, 0:512],
            )
            nc.scalar.dma_start(
                out=wx_sb[:, 512:D_H],
                in_=wxT[0:128, 512:D_H],
            )
            nc.gpsimd.dma_start(
                out=x_sbs[0][:, 0:CH],
                in_=xT[0:128, 0:CH],
            )
            for kb in range(1, KX):
                eng = nc.sync if kb % 2 == 1 else nc.scalar
                eng.dma_start(
                    out=wx_sb[:, kb * D_H:(kb + 1) * D_H],
                    in_=wxT[kb * 128:(kb + 1) * 128, :],
                )
                nc.gpsimd.dma_start(
                    out=x_sbs[0][:, kb * CH:(kb + 1) * CH],
                    in_=xT[kb * 128:(kb + 1) * 128, 0:CH],
                )
            for c in range(1, NCH):
                nc.scalar.dma_start(
                    out=x_sbs[c][:].rearrange("p (kb c) -> p kb c", c=CH),
                    in_=xT[:, c * CH:(c + 1) * CH].rearrange("(kb p) c -> p kb c", p=128),
                )
            # recurrent-phase loads: fp8 weights (1MB) then bf16 h chunks;
            # each h chunk is quantized to fp8 on DVE as soon as it lands.
            nc.scalar.dma_start(
                out=wh_sb[:],
                in_=whq[:, :].rearrange("(jb p) h -> p jb h", p=128),
            )
            for hc in range(NCH):
                nc.gpsimd.dma_start(
                    out=h_sb[hc][:],
                    in_=hT[:, hc * CH:(hc + 1) * CH].rearrange("(jb p) c -> p jb c", p=128),
                )
                nc.vector.tensor_scalar_mul(hq_sb[0][hc][:], h_sb[hc][:], S_H)
            # chunk 0: kb-outer across all 8 psum banks, so matmuls
            # start as soon as each wx k-block's DMA lands (no mid-group
            # stalls while the head loads stream in).
            ps0 = [
                psum_pool.tile([128, 2, CH], F32, name="ps0", tag="ps")
                for _ in range(KB // 2)
            ]
            for kb in range(KX):
                for p in range(KB):
                    nc.tensor.matmul(
                        ps0[p // 2][:, p % 2, :],
                        wx_sb[:, kb * D_H + p * 128: kb * D_H + (p + 1) * 128],
                        x_sbs[0][:, kb * CH:(kb + 1) * CH],
                        start=(kb == 0),
                        stop=(kb == KX - 1),
                    )
            for p in range(KB):
                # xw_pre = psum*(S_W*S_H) + b_pre on ACT (bias comes
                # pre-scaled from the host); pre-scaling lets the unfold
                # psum group add xw via an identity matmul with the tanh
                # descale folded into the activation input scale.
                nc.scalar.activation(
                    xw_sb[0][:, p, :], ps0[p // 2][:, p % 2, :], Ident,
                    bias=b_sb[:, p:p + 1], scale=float(S_W * S_H),
                )
            for c in range(1, NCH):
                x_sb = x_sbs[c]
                for p in range(KB):
                    ps1 = psum_pool.tile([128, 2, CH], F32, name="ps1", tag="ps")
                    for kb in range(KX):
                        nc.tensor.matmul(
                            ps1[:, 0, :],
                            wx_sb[:, kb * D_H + p * 128: kb * D_H + (p + 1) * 128],
                            x_sb[:, kb * CH:(kb + 1) * CH],
                            start=(kb == 0),
                            stop=(kb == KX - 1),
                        )
                    nc.scalar.activation(
                        xw_sb[c][:, p, :], ps1[:, 0, :], Ident,
                        bias=b_sb[:, p:p + 1], scale=float(S_W * S_H),
                    )

        # --- phase 2: unfold loop ---
        # hq holds S_H * h_t (physical h); psum = xw_pre + S_W*S_H*(W^T h)
        # via an identity matmul first in each accumulation group, so the
        # descale is just the tanh input scale (no DVE stt at all).
        fpool = ctx.enter_context(tc.tile_pool(name="fpool", bufs=2))
        zpool = ctx.enter_context(tc.tile_pool(name="zpool", bufs=3))
        alpha = 1.0 / (S_W * S_H)
        sigma = 1.0  # SBUF h_sb holds H_t = h_t / sigma
        deferred = []  # closures emitting DVE tail pieces (update/quantize)

        def flush(n_keep):
            while len(deferred) > n_keep:
                deferred.pop(0)()

        for t in range(UNFOLDS):
            last = t == UNFOLDS - 1
            par, nxt = t % 2, (t + 1) % 2
            upd = DT / (sigma * DECAY)    # coefficient on f for the H update
            qscale = S_H * sigma * DECAY  # h_{t+1} = sigma*0.9*H_{t+1}
            for c in range(NCH):
                f_ch = fpool.tile(
                    [128, KB, CH], BF16, name="f_ch", tag="f_ch", bufs=2
                )
                for q in range(KB // 2):
                    # groups 1-3: xw enters the psum via an identity matmul
                    # (cheap on PE); group 0: plain DVE add, balancing PE/DVE.
                    # q0 so the add sits at the DVE queue head for the chunk -
                    # its psum tile is WAR-needed by the next chunk's q0 MMs.
                    on_pe = q > 0
                    ps = psum_pool.tile([128, 2, CH], F32, name="ps", tag="ps")
                    for i in range(2):
                        p = 2 * q + i
                        if on_pe:
                            nc.tensor.matmul(
                                ps[:, i, :], id_sb[:], xw_sb[c][:, p, :],
                                start=True, stop=False,
                            )
                        for jj in range(KB // 2):
                            nc.tensor.matmul(
                                ps[:, i, :],
                                wh_sb[:, 2 * jj:2 * jj + 2, p * 128:(p + 1) * 128],
                                hq_sb[par][c][:, 2 * jj:2 * jj + 2, :],
                                start=(not on_pe and jj == 0),
                                stop=(jj == KB // 2 - 1),
                                perf_mode=mybir.MatmulPerfMode.DoubleRow,
                            )
                    if on_pe:
                        nc.scalar.activation(
                            f_ch[:, 2 * q:2 * q + 2, :], ps[:], Tanh,
                            scale=float(alpha),
                        )
                    else:
                        zt = zpool.tile([128, 2, CH], BF16, name="zt",
                                        tag="zt", bufs=3)
                        nc.vector.tensor_tensor(
                            zt[:], ps[:], xw_sb[c][:, 2 * q:2 * q + 2, :], ADD
                        )
                        nc.scalar.activation(
                            f_ch[:, 2 * q:2 * q + 2, :], zt[:], Tanh,
                            scale=float(alpha),
                        )
                    if last or q % 2 == 1:
                        # tail pieces for the slice just finished; defer so
                        # they land behind the next groups' work on DVE.  On
                        # the last step go per-group so the final chain after
                        # the last matmul is as short as possible.
                        k = q // 2
                        sl = slice(2 * q, 2 * q + 2) if last else slice(4 * k, 4 * k + 4)
                        def piece(c=c, sl=sl, k=k, upd=upd, qscale=qscale,
                                  f_ch=f_ch, par_nxt=nxt, last=last):
                            nc.vector.scalar_tensor_tensor(
                                h_sb[c][:, sl, :], f_ch[:, sl, :], float(upd),
                                h_sb[c][:, sl, :], op0=MUL, op1=ADD,
                            )
                            if last:
                                # raw half-chunk state out as soon as it's
                                # final; host multiplies by 0.9^6
                                nc.sync.dma_start(
                                    out=out[
                                        sl.start * 128:sl.stop * 128,
                                        c * CH:(c + 1) * CH,
                                    ].rearrange("(jb p) c -> p jb c", p=128),
                                    in_=h_sb[c][:, sl, :],
                                )
                            else:
                                nc.vector.tensor_scalar_mul(
                                    hq_sb[par_nxt][c][:, sl, :],
                                    h_sb[c][:, sl, :], float(qscale),
                                )
                        deferred.append(piece)
                    flush(1 if last else 2)
            sigma *= DECAY
        flush(0)
    return nc


_NC_CACHE = {}


def _get_nc() -> bass.Bass:
    if "nc" not in _NC_CACHE:
        _NC_CACHE["nc"] = build_nc()
    return _NC_CACHE["nc"]


def make_in_maps(x, h, fc_w, fc_b):
    x = np.asarray(x, dtype=np.float32)
    h = np.asarray(h, dtype=np.float32)
    fc_w = np.asarray(fc_w, dtype=np.float32)
    fc_b = np.asarray(fc_b, dtype=np.float32)
    xT = np.ascontiguousarray(x.T.astype(BF16NP))     # [D_IN, B] bf16
    hT = np.ascontiguousarray(h.T.astype(BF16NP))     # [D_H, B] bf16
    wxT = np.ascontiguousarray(fc_w[:, :D_IN].T.astype(BF16NP))  # [D_IN, D_H] bf16
    whT = np.ascontiguousarray(fc_w[:, D_IN:].T)      # [D_H, D_H]
    whq = np.clip(whT * S_W, -240.0, 240.0).astype(E4NP)
    bias = np.ascontiguousarray(fc_b.reshape(KB, 128).T) * (S_W * S_H)  # [128, KB]
    identity = np.eye(128, dtype=np.float32).astype(BF16NP)
    in_maps = []
    for i in range(N_CORES):
        sl = slice(i * BS, (i + 1) * BS)
        in_maps.append({
            "xT": np.ascontiguousarray(xT[:, sl]),
            "hT": np.ascontiguousarray(hT[:, sl]),
            "wxT": wxT,
            "whq": whq,
            "ident": identity,
            "bias": bias,
        })
    return in_maps


def gather_out(results):
    outT = np.concatenate([results[i]["out"] for i in range(N_CORES)], axis=1)
    # kernel stores the raw rescaled state H_6 = h_6 / 0.9^6 in bf16
    return np.ascontiguousarray(outT.T).astype(np.float32) * (DECAY ** UNFOLDS)


def kernel(x, h, fc_w, fc_b):
    nc = _get_nc()
    in_maps = make_in_maps(x, h, fc_w, fc_b)
    res = run_bass_kernel_spmd(nc, in_maps, list(range(N_CORES)))
    out = gather_out(res.results)
    return (out, out)


if __name__ == "__main__":
    rng = np.random.default_rng(0)
    x = rng.standard_normal((B, D_IN), dtype=np.float32)
    h = rng.standard_normal((B, D_H), dtype=np.float32)
    fc_w = rng.standard_normal((D_H, D_IN + D_H), dtype=np.float32) / np.sqrt(D_IN + D_H)
    fc_b = np.zeros((D_H,), dtype=np.float32)
    o, _ = kernel(x, h, fc_w, fc_b)
    print(o.shape, o.dtype)
